# revision 18
# baseline (speedup 1.0000x reference)
"""Cross-attention kernel for 8 TRN2 NeuronCores (SPMD, full-I/O contract).

Sharding: 8 cores = 2 batches x 4 head-groups (4 heads each).  Each core
computes its batch's attention for its 4 heads plus the row-sharded slice
of the output projection; the host sums the 4 partial projections per
batch (the "all-reduce") and adds bproj + bkv[D:] @ Wproj (the v-bias
passes through softmax additively since the weights sum to 1).

v3 structure (driven by HW traces):
  - Every matmul runs in (128,128) PE-tile mode: QK contracts over the
    full 128 partitions using zero-padded per-head q copies (qTz) with
    the kT t-tile shared as stationary.  No PE mode-switch drains.
  - The attention window is the ~73us ScalarE exp floor; all other PE
    work (q/k/v projections for later groups, output projection of
    finished halves) is emitted as "fill units" at fixed t-tile slots
    inside the window so the PE never idles and ScalarE stays saturated.
  - PSUM: psS holds S tiles double-buffered (4 banks) and doubles as the
    accumulator pool for fill units; psPV holds the two per-head PV
    accumulators (4 banks).
  - DMA: x first (q-proj gates the pipeline start), merged descriptors,
    l-half granularity; y leaves as bf16 per l-tile as soon as projected.
"""

import os
import sys

import numpy as np

B, L, T, D, H = 2, 2048, 2048, 1024, 16
HD = D // H
NCORES = 8
GROUPS = 4          # head-groups (tensor parallel)
DH_CORE = D // GROUPS  # 256 q/k/v dims per core
NEG_BIAS = -100000.0


def _ensure_paths():
    """Make axon site + concourse importable and provide antenv.axon_hooks
    (NTFF profile hook holder) if the image's antenv stub lacks it."""
    defaults = [
        "/root/.axon_site",
        "/root/.axon_site/_ro/trn_rl_repo",
        "/root/.axon_site/_ro/pypackages",
    ]
    for p in reversed(defaults):
        if os.path.isdir(p) and p not in sys.path:
            sys.path.insert(0, p)
    try:
        import antenv.axon_hooks  # noqa: F401
    except ImportError:
        import types

        mod = types.ModuleType("antenv.axon_hooks")
        mod._hook = None

        def set_axon_ntff_profile_hook(hook):
            mod._hook = hook

        def get_axon_ntff_profile_hook():
            return mod._hook

        mod.set_axon_ntff_profile_hook = set_axon_ntff_profile_hook
        mod.get_axon_ntff_profile_hook = get_axon_ntff_profile_hook
        import antenv

        antenv.axon_hooks = mod
        sys.modules["antenv.axon_hooks"] = mod
    import antenv.axon_hooks as ah

    if ah.get_axon_ntff_profile_hook() is None:
        try:
            from trn_agent_boot.trn_boot import _ntff_profile_via_ctypes

            hook = _ntff_profile_via_ctypes("/opt/axon/libaxon_pjrt.so")
            if hook is not None:
                ah.set_axon_ntff_profile_hook(hook)
        except Exception:
            pass


_ensure_paths()

_BUILD_CACHE = {}
LAST_RESULT = None


def build_bass(ntt):
    """Build the SPMD Bass program. ntt = number of 128-row tiles of the
    gathered+padded context length T_pad."""
    from concourse import bacc
    import concourse.bass as bass
    import concourse.mybir as mybir
    import concourse.tile as tile

    T_pad = ntt * 128
    bf = mybir.dt.bfloat16
    f32 = mybir.dt.float32
    EXP = mybir.ActivationFunctionType.Exp

    nc = bacc.Bacc(
        "TRN2",
        target_bir_lowering=False,
        debug=False,
        enable_asserts=False,
        num_devices=NCORES,
    )

    # ---- DRAM I/O (per-core shards, host-prepped, partition-major so every
    # DMA is 128 contiguous per-partition runs) ----
    xT_d = nc.dram_tensor("xT", [128, 2, 8, L // 2], bf, kind="ExternalInput").ap()
    ctxT_d = nc.dram_tensor("ctxT", [128, 8, T_pad], bf, kind="ExternalInput").ap()
    wq_d = nc.dram_tensor("wq", [128, 8, DH_CORE], bf, kind="ExternalInput").ap()
    wk_d = nc.dram_tensor("wk", [128, 8, DH_CORE], bf, kind="ExternalInput").ap()
    wv_d = nc.dram_tensor("wv", [128, 8, DH_CORE], bf, kind="ExternalInput").ap()
    wp_d = nc.dram_tensor("wp", [128, 2, D], bf, kind="ExternalInput").ap()
    bq_d = nc.dram_tensor("bq", [2, 128], f32, kind="ExternalInput").ap()
    bk_d = nc.dram_tensor("bk", [2, 128], f32, kind="ExternalInput").ap()
    mb_d = nc.dram_tensor("mb", [ntt, 128], f32, kind="ExternalInput").ap()
    y_d = nc.dram_tensor("y", [L, D], bf, kind="ExternalOutput").ap()

    SCALE = float(HD) ** -0.5
    HALF = L // 2          # 1024 columns per l-half
    NLT = HALF // 128      # 8 l-tiles per half
    tch = []
    t0 = 0
    while t0 < T_pad:
        tch.append((t0, min(512, T_pad - t0)))
        t0 += 512

    with tile.TileContext(nc) as tc:
        import contextlib

        ctx = contextlib.ExitStack()
        with ctx:
            singles = ctx.enter_context(tc.tile_pool(name="singles", bufs=1))
            psS = ctx.enter_context(tc.tile_pool(name="psS", bufs=2, space="PSUM"))
            psPV = ctx.enter_context(tc.tile_pool(name="psPV", bufs=2, space="PSUM"))
            ppool = ctx.enter_context(tc.tile_pool(name="ppool", bufs=5))
            npool = ctx.enter_context(tc.tile_pool(name="npool", bufs=4))
            ypool = ctx.enter_context(tc.tile_pool(name="ypool", bufs=4))

            # ---- ACT exp-table preload: tiny dummy exp with no deps ----
            warm = singles.tile([1, 16], f32)
            nc.vector.memset(warm, 0.0)
            warm2 = singles.tile([1, 16], f32)
            nc.scalar.activation(warm2, warm, EXP)

            # ---- resident inputs; DMA issue order = arrival order ----
            xT = singles.tile([128, 2, 8, HALF], bf)    # x^T (hf, k) tiles
            nc.sync.dma_start(out=xT[:, 0], in_=xT_d[:, 0])
            wq = singles.tile([128, 8, DH_CORE], bf)
            nc.sync.dma_start(out=wq, in_=wq_d)
            bq_sb = singles.tile([128, 2], f32)
            nc.sync.dma_start(out=bq_sb, in_=bq_d.rearrange("m p -> p m"))
            ctxT = singles.tile([128, 8, T_pad], bf)    # ctx'^T k-tiles
            nc.sync.dma_start(out=ctxT, in_=ctxT_d)
            wk = singles.tile([128, 8, DH_CORE], bf)
            nc.sync.dma_start(out=wk, in_=wk_d)
            wv = singles.tile([128, 8, DH_CORE], bf)
            nc.sync.dma_start(out=wv, in_=wv_d)
            bk_sb = singles.tile([128, 2], f32)
            nc.sync.dma_start(out=bk_sb, in_=bk_d.rearrange("m p -> p m"))
            mb_sb = singles.tile([128, ntt], f32)       # exp bias per t-tile
            nc.sync.dma_start(out=mb_sb, in_=mb_d.rearrange("t p -> p t"))
            nc.sync.dma_start(out=xT[:, 1], in_=xT_d[:, 1])
            wp = singles.tile([128, 2, D], bf)          # Wproj rows (2 k-tiles)
            nc.sync.dma_start(out=wp, in_=wp_d)

            # ---- PE warm-up during the DMA wait: HAM reaches K=8/8 and the
            # first real matmuls run at 2.4 GHz instead of 1.2 ----
            wsrc = singles.tile([128, 64], bf)
            nc.vector.memset(wsrc, 0.0)
            wps = psS.tile([128, 1024], f32, name="wps", tag="ps")
            NWARM = 160
            for i in range(NWARM):
                nc.tensor.matmul(
                    wps[0:64, 0:64], wsrc, wsrc,
                    start=(i == 0), stop=(i == NWARM - 1),
                )

            # ---- residents produced on device ----
            # qTz[p][h]: zero-padded per-head q^T (head h in rows h*64..+63)
            qTz = [[singles.tile([128, L], bf, name=f"qTz{p}{h}") for h in range(2)]
                   for p in range(2)]
            for p in range(2):
                nc.vector.memset(qTz[p][0][64:128, :], 0.0)
                nc.vector.memset(qTz[p][1][0:64, :], 0.0)
            kT = [singles.tile([128, T_pad], bf, name=f"kT{p}") for p in range(2)]
            v1 = singles.tile([128, ntt, 4, HD + 1], bf)
            nc.vector.memset(v1[:, :, :, HD : HD + 1], 1.0)
            outT = [singles.tile([128, L], bf, name=f"outT{p}") for p in range(2)]

            # ---- fill units (each: one psS grab + matmuls + DVE tail) ----
            def q_unit(hf, m):
                lo = hf * HALF
                acc = psS.tile([128, 1024], f32, name="qacc", tag="ps")
                for k in range(8):
                    for c in range(2):
                        nc.tensor.matmul(
                            acc[:, c * 512 : (c + 1) * 512],
                            wq[:, k, m * 128 : (m + 1) * 128],
                            xT[:, hf, k, c * 512 : (c + 1) * 512],
                            start=(k == 0),
                            stop=(k == 7),
                        )
                for h in range(2):
                    r0 = h * 64
                    nc.vector.tensor_scalar(
                        out=qTz[m][h][r0 : r0 + 64, lo : lo + HALF],
                        in0=acc[r0 : r0 + 64, :],
                        scalar1=bq_sb[r0 : r0 + 64, m : m + 1],
                        scalar2=SCALE,
                        op0=mybir.AluOpType.add,
                        op1=mybir.AluOpType.mult,
                    )

            def k_unit(m):
                acc = psS.tile([128, 1024], f32, name="kacc", tag="ps")
                for k in range(8):
                    for (tc0, tw) in tch:
                        nc.tensor.matmul(
                            acc[:, tc0 : tc0 + tw],
                            wk[:, k, m * 128 : (m + 1) * 128],
                            ctxT[:, k, tc0 : tc0 + tw],
                            start=(k == 0),
                            stop=(k == 7),
                        )
                nc.vector.tensor_scalar(
                    out=kT[m][:, 0:T_pad],
                    in0=acc[:, 0:T_pad],
                    scalar1=bk_sb[:, m : m + 1],
                    scalar2=None,
                    op0=mybir.AluOpType.add,
                )

            def v_unit(tt):
                pacc = psS.tile([128, 1024], f32, name="vacc", tag="ps")
                acc = pacc[:, 0:DH_CORE]
                for k in range(8):
                    nc.tensor.matmul(
                        acc,
                        ctxT[:, k, tt * 128 : (tt + 1) * 128],
                        wv[:, k, :],
                        start=(k == 0),
                        stop=(k == 7),
                    )
                for h in range(4):
                    nc.vector.tensor_copy(
                        v1[:, tt, h, 0:HD], acc[:, h * HD : (h + 1) * HD]
                    )

            def proj_unit(lt):
                l0 = lt * 128
                acc = psS.tile([128, 1024], f32, name="yacc", tag="ps")
                for p in range(2):
                    for nk in range(2):
                        nc.tensor.matmul(
                            acc[:, nk * 512 : (nk + 1) * 512],
                            outT[p][:, l0 : l0 + 128],
                            wp[:, p, nk * 512 : (nk + 1) * 512],
                            start=(p == 0),
                            stop=(p == 1),
                        )
                yt = ypool.tile([128, D], bf, tag="yt")
                nc.vector.tensor_copy(yt, acc)
                nc.sync.dma_start(out=y_d[l0 : l0 + 128, :], in_=yt)

            # ---- attention group with interleaved fill units ----
            def attn_group(p, hf, fills, final=False):
                lo = hf * HALF
                pv = [psPV.tile([128, 1024], f32, name=f"pv{h}", tag="pspv")
                      for h in range(2)]
                fq = list(fills)
                for tt in range(ntt):
                    Sr = [psS.tile([128, 1024], f32, name=f"S{h}", tag="ps")
                          for h in range(2)]
                    for h in range(2):
                        for lc in range(2):
                            nc.tensor.matmul(
                                Sr[h][:, lc * 512 : (lc + 1) * 512],
                                kT[p][:, tt * 128 : (tt + 1) * 128],
                                qTz[p][h][:, lo + lc * 512 : lo + (lc + 1) * 512],
                                start=True,
                                stop=True,
                            )
                    pt = [ppool.tile([128, 1024], bf, name=f"P{h2}", tag="P")
                          for h2 in range(2)]
                    for h in range(2):
                        nc.scalar.activation(
                            pt[h], Sr[h], EXP, bias=mb_sb[:, tt : tt + 1]
                        )
                    if fq and (tt % 2 == 1 or len(fq) >= ntt - tt):
                        fq.pop(0)()
                    for h in range(2):
                        for lc in range(2):
                            nc.tensor.matmul(
                                pv[h][0 : HD + 1, lc * 512 : (lc + 1) * 512],
                                v1[:, tt, p * 2 + h, :],
                                pt[h][:, lc * 512 : (lc + 1) * 512],
                                start=(tt == 0),
                                stop=(tt == ntt - 1),
                            )
                for f in fq:
                    f()
                # normalize: out^T[d, l] * (1 / sums[l]) -> bf16 resident
                for h in range(2):
                    srow = npool.tile([1, 1024], f32, name="srow", tag="srow")
                    if final:
                        # ScalarE is idle once the last exp retired
                        nc.scalar.copy(srow, pv[h][HD : HD + 1, :])
                    else:
                        nc.vector.tensor_copy(srow, pv[h][HD : HD + 1, :])
                    rec1 = npool.tile([1, 1024], f32, name="rec1", tag="rec1")
                    nc.vector.reciprocal_approx_fast(rec1, srow)
                    rec = npool.tile([64, 1024], f32, name="rec", tag="rec")
                    nc.gpsimd.partition_broadcast(rec, rec1)
                    nc.vector.tensor_mul(
                        outT[p][h * 64 : (h + 1) * 64, lo : lo + HALF],
                        pv[h][0:HD, :],
                        rec,
                    )

            # ---- program ----
            q_unit(0, 0)
            k_unit(0)
            for tt in range(ntt):
                v_unit(tt)
            attn_group(0, 0, [lambda: q_unit(0, 1), lambda: k_unit(1)])
            attn_group(1, 0, [lambda: q_unit(1, 0), lambda: q_unit(1, 1)])
            attn_group(0, 1, [(lambda i=i: proj_unit(i)) for i in range(0, 4)])
            attn_group(1, 1, [(lambda i=i: proj_unit(i)) for i in range(4, NLT)],
                       final=True)
            for lt in range(NLT, 2 * NLT):
                proj_unit(lt)

    nc.compile()
    return nc


def kernel(x, ctx, ctx_mask, Wq, bq, Wkv, bkv, Wproj, bproj):
    import ml_dtypes

    x = np.asarray(x, np.float32)
    ctx = np.asarray(ctx, np.float32)
    ctx_mask = np.asarray(ctx_mask)
    Wq = np.asarray(Wq, np.float32)
    bq = np.asarray(bq, np.float32)
    Wkv = np.asarray(Wkv, np.float32)
    bkv = np.asarray(bkv, np.float32)
    Wproj = np.asarray(Wproj, np.float32)
    bproj = np.asarray(bproj, np.float32)
    assert x.shape == (B, L, D) and ctx.shape == (B, T, D)

    bff = ml_dtypes.bfloat16

    # gather context by mask per batch; common padded length for SPMD
    idxs = [np.flatnonzero(ctx_mask[b]) for b in range(B)]
    tmax = max(1, max(len(i) for i in idxs))
    ntt = (tmax + 127) // 128
    T_pad = ntt * 128

    key = ntt
    if key not in _BUILD_CACHE:
        _BUILD_CACHE[key] = build_bass(ntt)
    nc = _BUILD_CACHE[key]

    in_maps = []
    for core in range(NCORES):
        b, g = core // GROUPS, core % GROUPS
        idx = idxs[b]
        tp = len(idx)
        ctxg = np.zeros((T_pad, D), np.float32)
        ctxg[:tp] = ctx[b][idx]
        mb = np.full(T_pad, NEG_BIAS, np.float32)
        mb[:tp] = 0.0
        s = slice(g * DH_CORE, (g + 1) * DH_CORE)
        # partition-major packing: [p, ...] with contiguous per-partition runs
        xTp = x[b].T.reshape(8, 128, 2, L // 2).transpose(1, 2, 0, 3)
        ctxp = ctxg.T.reshape(8, 128, T_pad).transpose(1, 0, 2)
        wqp = Wq[:, s].reshape(8, 128, DH_CORE).transpose(1, 0, 2)
        wkp = Wkv[:, s].reshape(8, 128, DH_CORE).transpose(1, 0, 2)
        wvp = (Wkv[:, D + g * DH_CORE : D + (g + 1) * DH_CORE]
               .reshape(8, 128, DH_CORE).transpose(1, 0, 2))
        wpp = Wproj[s, :].reshape(2, 128, D).transpose(1, 0, 2)
        in_maps.append(
            {
                "xT": np.ascontiguousarray(xTp).astype(bff),
                "ctxT": np.ascontiguousarray(ctxp).astype(bff),
                "wq": np.ascontiguousarray(wqp).astype(bff),
                "wk": np.ascontiguousarray(wkp).astype(bff),
                "wv": np.ascontiguousarray(wvp).astype(bff),
                "wp": np.ascontiguousarray(wpp).astype(bff),
                "bq": np.ascontiguousarray(bq[s].reshape(2, 128)),
                "bk": np.ascontiguousarray(bkv[s].reshape(2, 128)),
                "mb": np.ascontiguousarray(mb.reshape(ntt, 128)),
            }
        )

    from concourse.bass_utils import run_bass_kernel_spmd

    trace = bool(os.environ.get("KBENCH_TRACE"))
    res = run_bass_kernel_spmd(nc, in_maps, core_ids=list(range(NCORES)), trace=trace)
    global LAST_RESULT
    LAST_RESULT = res

    y = np.zeros((B, L, D), np.float32)
    for core in range(NCORES):
        y[core // GROUPS] += np.asarray(res.results[core]["y"], np.float32)
    # v-bias passes through softmax additively; bproj added once
    y += (bkv[D:] @ Wproj + bproj)[None, None, :]
    return y


# revision 19
# speedup vs baseline: 1.1807x; 1.1807x over previous
"""Cross-attention kernel for 8 TRN2 NeuronCores (SPMD, full-I/O contract).

Sharding: 8 cores = 2 batches x 4 head-groups (4 heads each).  Each core
computes its batch's attention for its 4 heads plus the row-sharded slice
of the output projection; the host sums the 4 partial projections per
batch (the "all-reduce") and adds bproj + bkv[D:] @ Wproj (the v-bias
passes through softmax additively since the weights sum to 1).

v3 structure (driven by HW traces):
  - Every matmul runs in (128,128) PE-tile mode: QK contracts over the
    full 128 partitions using zero-padded per-head q copies (qTz) with
    the kT t-tile shared as stationary.  No PE mode-switch drains.
  - The attention window is the ~73us ScalarE exp floor; all other PE
    work (q/k/v projections for later groups, output projection of
    finished halves) is emitted as "fill units" at fixed t-tile slots
    inside the window so the PE never idles and ScalarE stays saturated.
  - PSUM: psS holds S tiles double-buffered (4 banks) and doubles as the
    accumulator pool for fill units; psPV holds the two per-head PV
    accumulators (4 banks).
  - DMA: x first (q-proj gates the pipeline start), merged descriptors,
    l-half granularity; y leaves as bf16 per l-tile as soon as projected.
"""

import os
import sys

import numpy as np

B, L, T, D, H = 2, 2048, 2048, 1024, 16
HD = D // H
NCORES = 8
GROUPS = 4          # head-groups (tensor parallel)
DH_CORE = D // GROUPS  # 256 q/k/v dims per core
NEG_BIAS = -100000.0


def _ensure_paths():
    """Make axon site + concourse importable and provide antenv.axon_hooks
    (NTFF profile hook holder) if the image's antenv stub lacks it."""
    defaults = [
        "/root/.axon_site",
        "/root/.axon_site/_ro/trn_rl_repo",
        "/root/.axon_site/_ro/pypackages",
    ]
    for p in reversed(defaults):
        if os.path.isdir(p) and p not in sys.path:
            sys.path.insert(0, p)
    try:
        import antenv.axon_hooks  # noqa: F401
    except ImportError:
        import types

        mod = types.ModuleType("antenv.axon_hooks")
        mod._hook = None

        def set_axon_ntff_profile_hook(hook):
            mod._hook = hook

        def get_axon_ntff_profile_hook():
            return mod._hook

        mod.set_axon_ntff_profile_hook = set_axon_ntff_profile_hook
        mod.get_axon_ntff_profile_hook = get_axon_ntff_profile_hook
        import antenv

        antenv.axon_hooks = mod
        sys.modules["antenv.axon_hooks"] = mod
    import antenv.axon_hooks as ah

    if ah.get_axon_ntff_profile_hook() is None:
        try:
            from trn_agent_boot.trn_boot import _ntff_profile_via_ctypes

            hook = _ntff_profile_via_ctypes("/opt/axon/libaxon_pjrt.so")
            if hook is not None:
                ah.set_axon_ntff_profile_hook(hook)
        except Exception:
            pass


_ensure_paths()

_BUILD_CACHE = {}
LAST_RESULT = None


def build_bass(ntt):
    """Build the SPMD Bass program. ntt = number of 128-row tiles of the
    gathered+padded context length T_pad."""
    from concourse import bacc
    import concourse.bass as bass
    import concourse.mybir as mybir
    import concourse.tile as tile

    T_pad = ntt * 128
    bf = mybir.dt.bfloat16
    f32 = mybir.dt.float32
    EXP = mybir.ActivationFunctionType.Exp

    nc = bacc.Bacc(
        "TRN2",
        target_bir_lowering=False,
        debug=False,
        enable_asserts=False,
        num_devices=NCORES,
    )

    # ---- DRAM I/O (per-core shards, host-prepped, partition-major so every
    # DMA is 128 contiguous per-partition runs) ----
    xT_d = nc.dram_tensor("xT", [128, 2, 8, L // 2], bf, kind="ExternalInput").ap()
    ctxT_d = nc.dram_tensor("ctxT", [128, 8, T_pad], bf, kind="ExternalInput").ap()
    wq_d = nc.dram_tensor("wq", [128, 8, DH_CORE], bf, kind="ExternalInput").ap()
    wk_d = nc.dram_tensor("wk", [128, 8, DH_CORE], bf, kind="ExternalInput").ap()
    wv_d = nc.dram_tensor("wv", [128, 8, DH_CORE], bf, kind="ExternalInput").ap()
    wp_d = nc.dram_tensor("wp", [128, 2, D], bf, kind="ExternalInput").ap()
    bq_d = nc.dram_tensor("bq", [2, 128], f32, kind="ExternalInput").ap()
    bk_d = nc.dram_tensor("bk", [2, 128], f32, kind="ExternalInput").ap()
    mb_d = nc.dram_tensor("mb", [ntt, 128], f32, kind="ExternalInput").ap()
    y_d = nc.dram_tensor("y", [L, D], bf, kind="ExternalOutput").ap()

    SCALE = float(HD) ** -0.5
    HALF = L // 2          # 1024 columns per l-half
    NLT = HALF // 128      # 8 l-tiles per half
    tch = []
    t0 = 0
    while t0 < T_pad:
        tch.append((t0, min(512, T_pad - t0)))
        t0 += 512

    with tile.TileContext(nc) as tc:
        import contextlib

        ctx = contextlib.ExitStack()
        with ctx:
            singles = ctx.enter_context(tc.tile_pool(name="singles", bufs=1))
            psS = ctx.enter_context(tc.tile_pool(name="psS", bufs=2, space="PSUM"))
            psPV = ctx.enter_context(tc.tile_pool(name="psPV", bufs=2, space="PSUM"))
            ppool = ctx.enter_context(tc.tile_pool(name="ppool", bufs=5))
            npool = ctx.enter_context(tc.tile_pool(name="npool", bufs=4))
            ypool = ctx.enter_context(tc.tile_pool(name="ypool", bufs=4))

            # ---- ACT exp-table preload: tiny dummy exp with no deps ----
            warm = singles.tile([1, 16], f32)
            nc.vector.memset(warm, 0.0)
            warm2 = singles.tile([1, 16], f32)
            nc.scalar.activation(warm2, warm, EXP)

            # ---- resident inputs; DMA issue order = arrival order ----
            xT = singles.tile([128, 2, 8, HALF], bf)    # x^T (hf, k) tiles
            nc.sync.dma_start(out=xT[:, 0], in_=xT_d[:, 0])
            wq = singles.tile([128, 8, DH_CORE], bf)
            nc.sync.dma_start(out=wq, in_=wq_d)
            bq_sb = singles.tile([128, 2], f32)
            nc.sync.dma_start(out=bq_sb, in_=bq_d.rearrange("m p -> p m"))
            ctxT = singles.tile([128, 8, T_pad], bf)    # ctx'^T k-tiles
            nc.sync.dma_start(out=ctxT, in_=ctxT_d)
            wk = singles.tile([128, 8, DH_CORE], bf)
            nc.sync.dma_start(out=wk, in_=wk_d)
            wv = singles.tile([128, 8, DH_CORE], bf)
            nc.sync.dma_start(out=wv, in_=wv_d)
            bk_sb = singles.tile([128, 2], f32)
            nc.sync.dma_start(out=bk_sb, in_=bk_d.rearrange("m p -> p m"))
            mb_sb = singles.tile([128, ntt], f32)       # exp bias per t-tile
            nc.sync.dma_start(out=mb_sb, in_=mb_d.rearrange("t p -> p t"))
            nc.sync.dma_start(out=xT[:, 1], in_=xT_d[:, 1])
            wp = singles.tile([128, 2, D], bf)          # Wproj rows (2 k-tiles)
            nc.sync.dma_start(out=wp, in_=wp_d)

            # ---- PE warm-up during the DMA wait: HAM reaches K=8/8 and the
            # first real matmuls run at 2.4 GHz instead of 1.2 ----
            wsrc = singles.tile([128, 64], bf)
            nc.vector.memset(wsrc, 0.0)
            wps = psS.tile([128, 1024], f32, name="wps", tag="ps")
            NWARM = 160
            for i in range(NWARM):
                nc.tensor.matmul(
                    wps[0:64, 0:64], wsrc, wsrc,
                    start=(i == 0), stop=(i == NWARM - 1),
                )

            # ---- residents produced on device ----
            # qTz[p][h]: zero-padded per-head q^T (head h in rows h*64..+63)
            qTz = [[singles.tile([128, L], bf, name=f"qTz{p}{h}") for h in range(2)]
                   for p in range(2)]
            for p in range(2):
                nc.vector.memset(qTz[p][0][64:128, :], 0.0)
                nc.vector.memset(qTz[p][1][0:64, :], 0.0)
            kT = [singles.tile([128, T_pad], bf, name=f"kT{p}") for p in range(2)]
            v1 = singles.tile([128, ntt, 4, HD + 1], bf)
            nc.vector.memset(v1[:, :, :, HD : HD + 1], 1.0)
            outT = [singles.tile([128, L], bf, name=f"outT{p}") for p in range(2)]

            # ---- fill units (each: one psS grab + matmuls + DVE tail) ----
            def q_unit(hf, m):
                lo = hf * HALF
                acc = psS.tile([128, 1024], f32, name="qacc", tag="ps")
                for k in range(8):
                    for c in range(2):
                        nc.tensor.matmul(
                            acc[:, c * 512 : (c + 1) * 512],
                            wq[:, k, m * 128 : (m + 1) * 128],
                            xT[:, hf, k, c * 512 : (c + 1) * 512],
                            start=(k == 0),
                            stop=(k == 7),
                        )
                for h in range(2):
                    r0 = h * 64
                    nc.vector.tensor_scalar(
                        out=qTz[m][h][r0 : r0 + 64, lo : lo + HALF],
                        in0=acc[r0 : r0 + 64, :],
                        scalar1=bq_sb[r0 : r0 + 64, m : m + 1],
                        scalar2=SCALE,
                        op0=mybir.AluOpType.add,
                        op1=mybir.AluOpType.mult,
                    )

            def k_unit(m):
                acc = psS.tile([128, 1024], f32, name="kacc", tag="ps")
                for k in range(8):
                    for (tc0, tw) in tch:
                        nc.tensor.matmul(
                            acc[:, tc0 : tc0 + tw],
                            wk[:, k, m * 128 : (m + 1) * 128],
                            ctxT[:, k, tc0 : tc0 + tw],
                            start=(k == 0),
                            stop=(k == 7),
                        )
                nc.vector.tensor_scalar(
                    out=kT[m][:, 0:T_pad],
                    in0=acc[:, 0:T_pad],
                    scalar1=bk_sb[:, m : m + 1],
                    scalar2=None,
                    op0=mybir.AluOpType.add,
                )

            def v_unit(tt):
                pacc = psS.tile([128, 1024], f32, name="vacc", tag="ps")
                acc = pacc[:, 0:DH_CORE]
                for k in range(8):
                    nc.tensor.matmul(
                        acc,
                        ctxT[:, k, tt * 128 : (tt + 1) * 128],
                        wv[:, k, :],
                        start=(k == 0),
                        stop=(k == 7),
                    )
                for h in range(4):
                    nc.vector.tensor_copy(
                        v1[:, tt, h, 0:HD], acc[:, h * HD : (h + 1) * HD]
                    )

            def proj_unit(lt):
                l0 = lt * 128
                acc = psS.tile([128, 1024], f32, name="yacc", tag="ps")
                for p in range(2):
                    for nk in range(2):
                        nc.tensor.matmul(
                            acc[:, nk * 512 : (nk + 1) * 512],
                            outT[p][:, l0 : l0 + 128],
                            wp[:, p, nk * 512 : (nk + 1) * 512],
                            start=(p == 0),
                            stop=(p == 1),
                        )
                yt = ypool.tile([128, D], bf, tag="yt")
                nc.vector.tensor_copy(yt, acc)
                nc.sync.dma_start(out=y_d[l0 : l0 + 128, :], in_=yt)

            # ---- attention group with interleaved fill units ----
            def attn_group(p, hf, fills, final=False):
                lo = hf * HALF
                pv = [psPV.tile([128, 1024], f32, name=f"pv{h}", tag="pspv")
                      for h in range(2)]
                fq = list(fills)
                for tt in range(ntt):
                    Sr = [psS.tile([128, 1024], f32, name=f"S{h}", tag="ps")
                          for h in range(2)]
                    for h in range(2):
                        for lc in range(2):
                            nc.tensor.matmul(
                                Sr[h][:, lc * 512 : (lc + 1) * 512],
                                kT[p][:, tt * 128 : (tt + 1) * 128],
                                qTz[p][h][:, lo + lc * 512 : lo + (lc + 1) * 512],
                                start=True,
                                stop=True,
                            )
                    pt = [ppool.tile([128, 1024], bf, name=f"P{h2}", tag="P")
                          for h2 in range(2)]
                    for h in range(2):
                        nc.scalar.activation(
                            pt[h], Sr[h], EXP, bias=mb_sb[:, tt : tt + 1]
                        )
                    if fq and (tt % 2 == 1 or len(fq) >= ntt - tt):
                        fq.pop(0)()
                    for h in range(2):
                        for lc in range(2):
                            nc.tensor.matmul(
                                pv[h][0 : HD + 1, lc * 512 : (lc + 1) * 512],
                                v1[:, tt, p * 2 + h, :],
                                pt[h][:, lc * 512 : (lc + 1) * 512],
                                start=(tt == 0),
                                stop=(tt == ntt - 1),
                            )
                for f in fq:
                    f()
                # normalize: out^T[d, l] * (1 / sums[l]) -> bf16 resident
                for h in range(2):
                    srow = npool.tile([1, 1024], f32, name="srow", tag="srow")
                    if final:
                        # ScalarE is idle once the last exp retired
                        nc.scalar.copy(srow, pv[h][HD : HD + 1, :])
                    else:
                        nc.vector.tensor_copy(srow, pv[h][HD : HD + 1, :])
                    rec1 = npool.tile([1, 1024], f32, name="rec1", tag="rec1")
                    nc.vector.reciprocal_approx_fast(rec1, srow)
                    rec = npool.tile([64, 1024], f32, name="rec", tag="rec")
                    nc.gpsimd.partition_broadcast(rec, rec1)
                    nc.vector.tensor_mul(
                        outT[p][h * 64 : (h + 1) * 64, lo : lo + HALF],
                        pv[h][0:HD, :],
                        rec,
                    )

            # ---- program ----
            q_unit(0, 0)
            q_unit(0, 1)    # runs in the PE idle gap while ctxT still loads
            k_unit(0)
            for tt in range(ntt):
                v_unit(tt)
            attn_group(0, 0, [lambda: k_unit(1)])
            attn_group(1, 0, [lambda: q_unit(1, 0), lambda: q_unit(1, 1)])
            attn_group(0, 1, [(lambda i=i: proj_unit(i)) for i in range(0, 4)])
            attn_group(1, 1, [(lambda i=i: proj_unit(i)) for i in range(4, NLT)],
                       final=True)
            for lt in range(NLT, 2 * NLT):
                proj_unit(lt)

    nc.compile()
    return nc


def kernel(x, ctx, ctx_mask, Wq, bq, Wkv, bkv, Wproj, bproj):
    import ml_dtypes

    x = np.asarray(x, np.float32)
    ctx = np.asarray(ctx, np.float32)
    ctx_mask = np.asarray(ctx_mask)
    Wq = np.asarray(Wq, np.float32)
    bq = np.asarray(bq, np.float32)
    Wkv = np.asarray(Wkv, np.float32)
    bkv = np.asarray(bkv, np.float32)
    Wproj = np.asarray(Wproj, np.float32)
    bproj = np.asarray(bproj, np.float32)
    assert x.shape == (B, L, D) and ctx.shape == (B, T, D)

    bff = ml_dtypes.bfloat16

    # gather context by mask per batch; common padded length for SPMD
    idxs = [np.flatnonzero(ctx_mask[b]) for b in range(B)]
    tmax = max(1, max(len(i) for i in idxs))
    ntt = (tmax + 127) // 128
    T_pad = ntt * 128

    key = ntt
    if key not in _BUILD_CACHE:
        _BUILD_CACHE[key] = build_bass(ntt)
    nc = _BUILD_CACHE[key]

    in_maps = []
    for core in range(NCORES):
        b, g = core // GROUPS, core % GROUPS
        idx = idxs[b]
        tp = len(idx)
        ctxg = np.zeros((T_pad, D), np.float32)
        ctxg[:tp] = ctx[b][idx]
        mb = np.full(T_pad, NEG_BIAS, np.float32)
        mb[:tp] = 0.0
        s = slice(g * DH_CORE, (g + 1) * DH_CORE)
        # partition-major packing: [p, ...] with contiguous per-partition runs
        xTp = x[b].T.reshape(8, 128, 2, L // 2).transpose(1, 2, 0, 3)
        ctxp = ctxg.T.reshape(8, 128, T_pad).transpose(1, 0, 2)
        wqp = Wq[:, s].reshape(8, 128, DH_CORE).transpose(1, 0, 2)
        wkp = Wkv[:, s].reshape(8, 128, DH_CORE).transpose(1, 0, 2)
        wvp = (Wkv[:, D + g * DH_CORE : D + (g + 1) * DH_CORE]
               .reshape(8, 128, DH_CORE).transpose(1, 0, 2))
        wpp = Wproj[s, :].reshape(2, 128, D).transpose(1, 0, 2)
        in_maps.append(
            {
                "xT": np.ascontiguousarray(xTp).astype(bff),
                "ctxT": np.ascontiguousarray(ctxp).astype(bff),
                "wq": np.ascontiguousarray(wqp).astype(bff),
                "wk": np.ascontiguousarray(wkp).astype(bff),
                "wv": np.ascontiguousarray(wvp).astype(bff),
                "wp": np.ascontiguousarray(wpp).astype(bff),
                "bq": np.ascontiguousarray(bq[s].reshape(2, 128)),
                "bk": np.ascontiguousarray(bkv[s].reshape(2, 128)),
                "mb": np.ascontiguousarray(mb.reshape(ntt, 128)),
            }
        )

    from concourse.bass_utils import run_bass_kernel_spmd

    trace = bool(os.environ.get("KBENCH_TRACE"))
    res = run_bass_kernel_spmd(nc, in_maps, core_ids=list(range(NCORES)), trace=trace)
    global LAST_RESULT
    LAST_RESULT = res

    y = np.zeros((B, L, D), np.float32)
    for core in range(NCORES):
        y[core // GROUPS] += np.asarray(res.results[core]["y"], np.float32)
    # v-bias passes through softmax additively; bproj added once
    y += (bkv[D:] @ Wproj + bproj)[None, None, :]
    return y
